# revision 21
# baseline (speedup 1.0000x reference)
"""Trainium2 Bass kernel for group-quant (fake int8, V=64) + Linear.

reference math (per row of x):
    absmax over feature-groups of 64 -> delta = max(2*absmax/254, 1e-5)
    xq = clip(round(x/delta), -127, 127) * delta      (fake quant)
    out = xq @ W.T + b

Sharding: data-parallel on tokens across 8 cores (1024 rows each);
W (pre-transposed to [in,out] and cast fp16 on host) + b replicated.

Device pipeline per core (t-tile = 128 token rows, 8 per core):
  per t-tile: load x (gpsimd queue) -> vector-only fake-quant chain
  (absmax reduce, delta, recip, y=x*recip, RNE round via +/-1.5*2^23,
  dequant to fp16) -> 32 PE transposes (xh.T @ I into fp16 PSUM) each
  evacuated to the per-t x~^T SBUF tile by the otherwise-idle ACT
  engine.  No DRAM bounce, no XBAR DMA transposes.  Matmuls run
  oc-pass-outer so W streams from HBM exactly once (fp16 eighths on
  the sync queue); the first two oc-passes are interleaved over t so
  the PE has work as soon as tile 0 is transposed.  Emission is
  software-pipelined (quant t+2 / mm t / transpose t+2) so no engine
  queue head-blocks on a cross-engine dependency.
"""

import numpy as np

import concourse.bass as bass
import concourse.mybir as mybir
import concourse.tile as tile
from concourse.bass_utils import run_bass_kernel_spmd

N_CORES = 8
MAGIC = 1.5 * 2.0**23      # fp32 round-to-nearest-even constant
QSCALE = 1.0 / 127.0       # 2/(qmax-qmin) with qmax=127, qmin=-127
DELTA_MIN = 1e-5


def _split_multiwait(nc):
    """This walrus build allows at most ONE sync wait per instruction
    ("Too many sync wait commands", CoreV3GenImpl setupSyncWait) and none
    on Drain. Tile freely attaches several waits to one instruction, so
    post-process: move excess waits onto single-wait NoOps inserted just
    before the instruction on the same engine queue (semantics identical —
    the queue stalls at the nop instead of at the instruction)."""
    nid = 0
    for fn in nc.m.functions:
        for bb in fn.blocks:
            insts = list(bb.instructions)
            out = []
            changed = False
            for inst in insts:
                si = inst.sync_info
                waits = list(si.on_wait) if si is not None and si.on_wait else []
                limit = 0 if type(inst).__name__ == "InstDrain" else 1
                if len(waits) > limit:
                    changed = True
                    keep = waits[len(waits) - limit :] if limit else []
                    for w in waits[: len(waits) - limit]:
                        nid += 1
                        out.append(
                            mybir.InstNoOp(
                                name=f"WSPLIT-{nid}",
                                engine=inst.engine,
                                bass_nofuse=True,
                                ins=[],
                                outs=[],
                                sync_info=mybir.SyncInfo(on_wait=[w], on_update=[]),
                            )
                        )
                    si.on_wait = keep
                out.append(inst)
            if changed:
                try:
                    bb.instructions = out
                except Exception:
                    bb.instructions[:] = out


def build(T=1024, K=4096, O=4096, V=64, OC=512, split=True,
          ilv=2, wbufs=17, WE=8, trp=4, warm=0):
    f32, f16 = mybir.dt.float32, mybir.dt.float16
    P = 128
    G = K // V                 # quant groups per row
    KT = K // P                # contraction tiles
    NOC = O // OC              # output chunks
    KE = KT // WE              # k-tiles per W-load eighth
    NT = T // P                # token tiles per core
    assert NT * P == T

    nc = bass.Bass()
    x = nc.dram_tensor("x", [T, K], f32, kind="ExternalInput")
    wt = nc.dram_tensor("wt", [NOC, P, KT * OC], f16, kind="ExternalInput")
    bvec = nc.dram_tensor("b", [O], f32, kind="ExternalInput")
    ident = nc.dram_tensor("ident", [P, P], f16, kind="ExternalInput")
    out = nc.dram_tensor("out", [T, O], f32, kind="ExternalOutput")

    mult = mybir.AluOpType.mult
    add = mybir.AluOpType.add
    sub = mybir.AluOpType.subtract
    amax_op = mybir.AluOpType.max

    with tile.TileContext(nc) as tc:
        with (
            tc.tile_pool(name="xq", bufs=2) as pool_x,
            tc.tile_pool(name="xh", bufs=2) as pool_xh,
            tc.tile_pool(name="st", bufs=3) as pool_s,
            tc.tile_pool(name="xt", bufs=1) as pool_xt,
            tc.tile_pool(name="w", bufs=wbufs) as pool_w,
            tc.tile_pool(name="bias", bufs=1) as pool_b,
            tc.tile_pool(name="o", bufs=2) as pool_o,
            tc.tile_pool(name="id", bufs=1) as pool_id,
            tc.tile_pool(name="ps", bufs=4, space="PSUM") as pool_ps,
            tc.tile_pool(name="pt", bufs=trp, space="PSUM") as pool_pt,
        ):
            xT = [None] * NT           # per-t transposed tiles [P, KT, P]
            xh = [None] * NT           # per-t quantized fp16 tiles [P, K]
            wq = {}                    # (oc, e) -> W eighth tile [P, KE, OC]

            def load_w_eighth(oc, e):
                wqt = pool_w.tile([P, KE, OC], f16, tag="w", name=f"w{oc}_{e}")
                nc.sync.dma_start(
                    out=wqt.rearrange("p ke o -> p (ke o)"),
                    in_=wt[oc][:, e * KE * OC : (e + 1) * KE * OC],
                )
                wq[(oc, e)] = wqt

            def quant_load(t, halves=False):
                xt_ = pool_x.tile([P, K], f32, tag="xq", name=f"x{t}")
                if halves:
                    h = K // 2
                    nc.gpsimd.dma_start(
                        out=xt_[:, 0:h], in_=x[t * P : (t + 1) * P, 0:h])
                    nc.gpsimd.dma_start(
                        out=xt_[:, h:K], in_=x[t * P : (t + 1) * P, h:K])
                else:
                    nc.gpsimd.dma_start(out=xt_[:], in_=x[t * P : (t + 1) * P, :])
                return xt_

            def quant_rest(t, xt_, nch=1):
                # fake-quant the t-tile; nch>1 runs the chain per K-chunk so
                # the head tiles start before their full x load lands
                xh_t = pool_xh.tile([P, K], f16, tag="xh")
                kc, gc = K // nch, G // nch
                for c in range(nch):
                    xs_ = xt_[:, c * kc : (c + 1) * kc]
                    x3 = xs_.rearrange("p (g v) -> p g v", v=V)
                    amax = pool_s.tile([P, gc], f32, tag="amax")
                    nc.vector.tensor_reduce(
                        out=amax[:], in_=x3, axis=mybir.AxisListType.X,
                        op=amax_op, apply_absolute_value=True,
                    )
                    delta = pool_s.tile([P, gc], f32, tag="delta")
                    nc.vector.tensor_scalar(
                        out=delta[:], in0=amax[:],
                        scalar1=QSCALE, scalar2=DELTA_MIN, op0=mult, op1=amax_op,
                    )
                    recip = pool_s.tile([P, gc], f32, tag="recip")
                    nc.vector.reciprocal(out=recip[:], in_=delta[:])
                    # y = x / delta, in place (broadcast recip over groups)
                    nc.vector.tensor_tensor(
                        out=x3, in0=x3,
                        in1=recip[:, :, None].to_broadcast((P, gc, V)), op=mult,
                    )
                    # q = round(y): exact fp32 RNE via +/-MAGIC; |y| <= 127 so
                    # the integer result is exact in fp16
                    xhs = xh_t[:, c * kc : (c + 1) * kc]
                    nc.vector.tensor_scalar(
                        out=xhs, in0=xs_,
                        scalar1=MAGIC, scalar2=MAGIC, op0=add, op1=sub,
                    )
                    # xq = q * delta, in place fp16
                    xh3 = xhs.rearrange("p (g v) -> p g v", v=V)
                    nc.vector.tensor_tensor(
                        out=xh3, in0=xh3,
                        in1=delta[:, :, None].to_broadcast((P, gc, V)), op=mult,
                    )
                xh[t] = xh_t

            def transpose_tile(t, itile):
                # PE transpose of each [128,128] k-slice into fp16 PSUM (8 per
                # bank), then ACT (scalar engine) evacuates a whole bank at a
                # time to the SBUF x~^T tile
                xT[t] = pool_xt.tile([P, KT, P], f16, tag=f"xT{t}", name=f"xT{t}")
                for k0 in range(0, KT, 8):
                    pst = pool_pt.tile([P, 8, P], f16, tag="pt")
                    for j in range(8):
                        nc.tensor.transpose(
                            pst[:, j, :],
                            xh[t][:, (k0 + j) * P : (k0 + j + 1) * P], itile[:]
                        )
                    nc.scalar.activation(
                        out=xT[t][:, k0 : k0 + 8, :], in_=pst[:],
                        func=mybir.ActivationFunctionType.Copy,
                    )

            def mm_group(oc, t, btile):
                ps = pool_ps.tile([P, OC], f32, tag="ps")
                for k in range(KT):
                    nc.tensor.matmul(
                        ps[:],
                        xT[t][:, k, :],
                        wq[(oc, k // KE)][:, k % KE, :],
                        start=(k == 0),
                        stop=(k == KT - 1),
                    )
                ot = pool_o.tile([P, OC], f32, tag="o")
                nc.vector.tensor_tensor(
                    out=ot[:], in0=ps[:],
                    in1=btile[:, oc * OC : (oc + 1) * OC], op=add,
                )
                nc.gpsimd.dma_start(
                    out=out[t * P : (t + 1) * P, oc * OC : (oc + 1) * OC], in_=ot[:]
                )

            # ---- emission (software-pipelined across engines) ----
            xt_tiles = [None] * NT
            xt_tiles[0] = quant_load(0, halves=True)
            xt_tiles[1] = quant_load(1, halves=True)
            itile = pool_id.tile([P, P], f16, tag="id", name="ident")
            nc.sync.dma_start(out=itile[:], in_=ident[:, :])
            for oc in range(ilv):
                for e in range(WE):
                    load_w_eighth(oc, e)
            btile = pool_b.tile([P, O], f32, tag="bias", name="bias")
            bsl = bvec[0:O]
            b_bcast = bass.AP(tensor=bsl.tensor, offset=bsl.offset,
                              ap=[[0, P], *bsl.ap])
            nc.sync.dma_start(out=btile[:], in_=b_bcast)
            quant_rest(0, xt_tiles[0], nch=2)
            quant_rest(1, xt_tiles[1], nch=2)
            # HAM warm-up: dummy PE transposes of the identity (never read)
            # keep the PE activity window hot through the quant head so the
            # real matmul stream starts at full clock
            for wi in range(warm):
                pstw = pool_pt.tile([P, 8, P], f16, tag="pt", name=f"wm{wi}")
                nc.tensor.transpose(pstw[:, 0, :], itile[:], itile[:])
            transpose_tile(0, itile)
            transpose_tile(1, itile)
            # fill: first ilv oc-passes interleaved over t; quant/transpose of
            # tile t+2 rides along; W prefetch for later passes trickles in
            pf = [(oc, e) for oc in range(ilv, NOC) for e in range(WE)]
            pfi = 0
            for t in range(NT):
                if t + 2 < NT:
                    xt_tiles[t + 2] = quant_load(t + 2)
                    quant_rest(t + 2, xt_tiles[t + 2])
                for oc in range(ilv):
                    mm_group(oc, t, btile)
                if t + 2 < NT:
                    transpose_tile(t + 2, itile)
                npf = 8 if t > 1 else 0
                for _ in range(npf):
                    if pfi < len(pf):
                        load_w_eighth(*pf[pfi]); pfi += 1
            # steady state: remaining oc-passes
            for oc in range(ilv, NOC):
                for t in range(NT):
                    mm_group(oc, t, btile)
                    if pfi < len(pf):
                        load_w_eighth(*pf[pfi]); pfi += 1
    if split:
        _split_multiwait(nc)
    return nc


_CACHED = {}

# test-harness knobs (kernel() defaults are what the grader uses)
TRACE = False
LAST_RESULT = None
BUILD_KW = {}


def _get_nc(shape_key):
    key = (shape_key, tuple(sorted(BUILD_KW.items())))
    if key not in _CACHED:
        T, K, O = shape_key
        _CACHED[key] = build(T=T, K=K, O=O, **BUILD_KW)
    return _CACHED[key]


def pack_w(W: np.ndarray, OC: int = 512, P: int = 128) -> np.ndarray:
    # [out,in] -> W^T [in,out] fp16, packed [NOC, P, KT*OC] so each per-core
    # o-chunk W load is one fully contiguous DMA
    K, O = W.shape[1], W.shape[0]
    KT, NOC = K // P, O // OC
    wt = np.ascontiguousarray(W.T).astype(np.float16)         # [K, O]
    z = wt.reshape(KT, P, NOC, OC).transpose(2, 1, 0, 3)      # [NOC, P, KT, OC]
    return np.ascontiguousarray(z.reshape(NOC, P, KT * OC))


def kernel(x: np.ndarray, W: np.ndarray, b: np.ndarray) -> np.ndarray:
    global LAST_RESULT
    n, k = x.shape               # 8192, 4096
    o = W.shape[0]               # 4096
    assert n % N_CORES == 0
    tpc = n // N_CORES
    nc = _get_nc((tpc, k, o))

    wt = pack_w(W)
    b32 = np.ascontiguousarray(b.astype(np.float32))
    ident = np.eye(128, dtype=np.float16)
    xs = np.ascontiguousarray(x.astype(np.float32)).reshape(N_CORES, tpc, k)
    in_maps = [{"x": xs[i], "wt": wt, "b": b32, "ident": ident}
               for i in range(N_CORES)]
    res = run_bass_kernel_spmd(nc, in_maps, list(range(N_CORES)), trace=TRACE)
    LAST_RESULT = res
    return np.concatenate([res.results[i]["out"] for i in range(N_CORES)], axis=0)
